# revision 1
# baseline (speedup 1.0000x reference)
"""ClockworkRNN forward kernel for 8 Trainium2 NeuronCores.

Strategy: data-parallel over batch (64 -> 8 per core).  Everything on-chip is
kept "d-major": the recurrent state H lives as [128 partitions(d within
group), 8 groups * 8 batch] so the per-step tanh is one cheap activation and
the clock matmuls use the natural cw layout as stationary weights.

Per core:
  - projection x = X @ W + b computed with bf16 matmuls (W chunks stationary,
    X^T streamed), scattered into a per-step record buffer xrec[:, t*64+g*8+b]
  - 512-step scan; step t updates groups 0..m where m = min(trailing zeros of
    t, 7).  xt is injected into PSUM with an identity matmul (start=True),
    clock matmuls accumulate on top, one tanh covers all active groups and a
    second covers the carried (inactive) ones.
  - output written to DRAM in scratch layout [128(dg), T, 8(g)*8(b)]; the
    host reshapes to [B, T, D] (free - not on the device clock).
"""

import sys

if "/opt/trn_rl_repo" not in sys.path:
    sys.path.insert(0, "/opt/trn_rl_repo")

import numpy as np
import ml_dtypes

import concourse.bass as bass
import concourse.tile as tile
from concourse import bacc, mybir
from concourse import bass_utils
from concourse.masks import make_identity

BF16 = ml_dtypes.bfloat16
N_CORES = 8
B, T, IN, D = 64, 512, 512, 1024
N = 128          # units per clock group
G = 8            # number of clock groups
BL = B // N_CORES  # batch per core
KCH = IN // 128  # contraction chunks for the projection

_CACHE = {}


def _m_of(t: int) -> int:
    """Highest active group index at step t (prefix 0..m updates)."""
    if t == 0:
        return G - 1
    return min((t & -t).bit_length() - 1, G - 1)


def _pair(i: int, k: int) -> int:
    """Index of chunk k of cw_i in the packed CW buffer."""
    return i * (i + 1) // 2 + k


def build_nc():
    nc = bacc.Bacc("TRN2", target_bir_lowering=False, debug=False,
                   num_devices=N_CORES)

    XT = nc.dram_tensor("XT", [IN, BL * T], mybir.dt.bfloat16,
                        kind="ExternalInput")
    Wt = nc.dram_tensor("Wt", [IN, D], mybir.dt.bfloat16,
                        kind="ExternalInput")
    CW = nc.dram_tensor("CW", [N, 36 * N], mybir.dt.bfloat16,
                        kind="ExternalInput")
    BIAS = nc.dram_tensor("BIAS", [N, G], mybir.dt.float32,
                          kind="ExternalInput")
    OUT = nc.dram_tensor("OUT", [N, T, G * BL], mybir.dt.float32,
                         kind="ExternalOutput")

    f32 = mybir.dt.float32
    bf16 = mybir.dt.bfloat16
    Tanh = mybir.ActivationFunctionType.Tanh

    with tile.TileContext(nc) as tc:
        with (
            tc.tile_pool(name="const", bufs=1) as const,
            tc.tile_pool(name="hpool", bufs=3) as hpool,
            tc.tile_pool(name="stage", bufs=3) as spool,
            tc.tile_pool(name="projp", bufs=2, space="PSUM") as ppool,
            tc.tile_pool(name="scanp", bufs=2, space="PSUM") as pspool,
        ):
            # ---- persistent SBUF state ----
            xt_sb = const.tile([128, KCH, BL * T], bf16)     # X^T
            w_sb = const.tile([128, KCH, D], bf16)           # W chunks
            cw_sb = const.tile([128, 36 * N], bf16)          # packed cw chunks
            bias_sb = const.tile([128, G], f32)
            ident = const.tile([128, 128], bf16)
            xrec = const.tile([128, T * G * BL], bf16)       # per-step records

            nc.sync.dma_start(out=xt_sb,
                              in_=XT.rearrange("(k p) c -> p k c", p=128))
            nc.sync.dma_start(out=w_sb,
                              in_=Wt.rearrange("(k p) d -> p k d", p=128))
            nc.sync.dma_start(out=cw_sb, in_=CW[:, :])
            nc.sync.dma_start(out=bias_sb, in_=BIAS[:, :])
            make_identity(nc, ident)

            # ---- projection: xrec[:, t*64 + g*8 + b] = (X W + b)^T ----
            for g in range(G):
                nt = T >> g                       # t' count for this group
                nb = min(8, 512 // nt)            # batches per psum tile
                ntile = BL // nb
                s = 1 << g
                xt_v = xt_sb.rearrange(
                    "p k (b tq s) -> p k b tq s", b=BL, s=s)
                xr_v = xrec.rearrange(
                    "p (tq s g b) -> p tq s g b", s=s, g=G, b=BL)
                for it in range(ntile):
                    b0 = it * nb
                    cols = nb * nt
                    psum = ppool.tile([128, 512], f32, tag="proj")
                    for k in range(KCH):
                        rhs = xt_v[:, k, b0:b0 + nb, :, 0]
                        nc.tensor.matmul(
                            psum[:, :cols].rearrange(
                                "p (b t) -> p b t", b=nb),
                            lhsT=w_sb[:, k, g * N:(g + 1) * N],
                            rhs=rhs,
                            start=(k == 0), stop=(k == KCH - 1),
                        )
                    dest = xr_v[:, :, 0, g, b0:b0 + nb].rearrange(
                        "p t b -> p b t")
                    nc.vector.tensor_scalar_add(
                        out=dest,
                        in0=psum[:, :cols].rearrange("p (b t) -> p b t", b=nb),
                        scalar1=bias_sb[:, g:g + 1],
                    )

            # ---- scan ----
            h_prev = hpool.tile([128, G * BL], bf16, tag="H")
            nc.vector.memset(h_prev, 0.0)

            stg = None
            for t in range(T):
                m = _m_of(t)
                act = BL * (m + 1)
                ps = pspool.tile([128, G * BL], f32, tag="ps")

                # xt -> psum (identity matmul, clears + seeds the bank)
                nc.tensor.matmul(
                    ps[:, 0:act], lhsT=ident,
                    rhs=xrec[:, t * G * BL: t * G * BL + act],
                    start=True, stop=False, skip_group_check=True,
                )
                # clock matmuls accumulate
                for i in range(m + 1):
                    for k in range(i + 1):
                        p = _pair(i, k)
                        nc.tensor.matmul(
                            ps[:, BL * i: BL * (i + 1)],
                            lhsT=cw_sb[:, p * N:(p + 1) * N],
                            rhs=h_prev[:, BL * k: BL * (k + 1)],
                            start=False, stop=(k == i),
                            skip_group_check=True,
                        )

                h_new = hpool.tile([128, G * BL], bf16, tag="H")
                nc.scalar.activation(h_new[:, 0:act], ps[:, 0:act], Tanh)
                if m < G - 1:
                    nc.scalar.activation(
                        h_new[:, act:], h_prev[:, act:], Tanh)

                if t % 4 == 0:
                    stg = spool.tile([128, 4, G * BL], f32, tag="stg")
                nc.vector.tensor_copy(stg[:, t % 4, :], h_new)
                if t % 4 == 3:
                    nc.sync.dma_start(out=OUT[:, t - 3:t + 1, :], in_=stg)

                h_prev = h_new

    nc.compile()
    return nc


def _prep_in_maps(X, W, b, cws):
    cw_pack = np.concatenate(
        [cws[i][k * N:(k + 1) * N, :] for i in range(G) for k in range(i + 1)],
        axis=1).astype(BF16)                       # [128, 4608]
    w_in = W.astype(BF16)
    bias_in = np.ascontiguousarray(b.reshape(G, N).T.astype(np.float32))
    in_maps = []
    for c in range(N_CORES):
        xc = X[c * BL:(c + 1) * BL]                # [BL, T, IN]
        xt_in = np.ascontiguousarray(
            xc.transpose(2, 0, 1).reshape(IN, BL * T)).astype(BF16)
        in_maps.append({
            "XT": xt_in, "Wt": w_in, "CW": cw_pack, "BIAS": bias_in,
        })
    return in_maps


def _assemble(results):
    out = np.empty((B, T, D), np.float32)
    for c in range(N_CORES):
        o = results[c]["OUT"]                      # [128, T, 64]
        out[c * BL:(c + 1) * BL] = (
            o.reshape(N, T, G, BL).transpose(3, 1, 2, 0).reshape(BL, T, D))
    return out


def kernel(X, W, b, cw0, cw1, cw2, cw3, cw4, cw5, cw6, cw7):
    X = np.asarray(X, np.float32)
    W = np.asarray(W, np.float32)
    b = np.asarray(b, np.float32)
    cws = [np.asarray(c, np.float32)
           for c in (cw0, cw1, cw2, cw3, cw4, cw5, cw6, cw7)]

    if "nc" not in _CACHE:
        _CACHE["nc"] = build_nc()
    nc = _CACHE["nc"]

    in_maps = _prep_in_maps(X, W, b, cws)
    res = bass_utils.run_bass_kernel_spmd(
        nc, in_maps, core_ids=list(range(N_CORES)))
    return _assemble(res.results)
